# revision 6
# baseline (speedup 1.0000x reference)
"""ClassWeightedModalDownSampler Trainium2 kernel — packed-digit histogram.

labels [4,1024,2048] (0..19), class_weights [20] f32, dsf=8 ->
modes [4,128,256]: per 8x8 patch argmax_c(hist_c * w_c), first-index ties.

Scheme (8 cores, 64 patch rows each):
  host:  V[p, n] = 2^(8*l - 100) as bf16 (exact powers of two), p = w mod 128,
         n = r*1024 + wc*64 + prow.
  planes: 7 planes of 3 classes {3g, 3g+1, 3g+2} built on DVE at 4x:
         P_g = min(V, 2^(8*(3g+2)-100)) * 2^(100-24g): in-window pixels give
         digits {1, 2^8, 2^16}; below-window pixels leave <=2^-8 crumbs;
         above-window pixels CLAMP onto the top digit (corrected later).
  PE:    per plane one pooling pass: block-ones lhsT contracts each patch's
         8 q-columns, 8 accumulating matmuls contract the 8 rows ->
         PSUM [128, 1024]: partition 16g+pi holds c0 + c1*256 + c2'*65536,
         with c2' = c2 + T_g (T_g = # pixels with class >= 3g+3).
  tail:  digit peel (int32 casts kill crumbs); T_g = digit-sum of plane g+1
         (telescoping), fetched by a 16-partition DMA shift; per-partition
         E = 64*w*c - cls on ACT; max across digits, stripe folds, decode.
"""

import numpy as np
import ml_dtypes

import concourse.bass as bass
import concourse.mybir as mybir
import concourse.tile as tile
from concourse import bacc
from concourse.bass_utils import run_bass_kernel_spmd

NCORES = 8
B, H, W = 4, 1024, 2048
DSF = 8
NCLS = 20
GH, GW = H // DSF, W // DSF
ROWS = (B * H) // NCORES     # 512 label rows per core
PROWS = ROWS // DSF          # 64 patch rows per core
P = 128
WC = W // P                  # 16 column chunks of 128
FREE = WC * ROWS             # 8192 free positions per partition
NPL = 7                      # planes of 3 classes (plane 6: 18, 19, dummy)

_DT = mybir.dt

TRACE = False
LAST_RESULTS = None


def _aux_arrays(class_weights: np.ndarray):
    # lhsT blocks: 7 matrices [128,128] bf16; block g has ones at
    # (p, 16*g + p//8): contracts each patch's 8 q-columns into
    # output partition 16*g + patch-index.
    lhst = np.zeros((P, NPL * P), dtype=np.float32)
    for g in range(NPL):
        for p in range(P):
            lhst[p, g * P + 16 * g + p // 8] = 1.0
    lhst_bf = lhst.astype(ml_dtypes.bfloat16)

    # per-partition E-encode constants: partition p = 16*g + pi, digit d
    # -> class c = 3g + d: scale 64*w_c, bias -c. Dummy slots lose the max.
    w = np.asarray(class_weights, dtype=np.float32)
    wsc = np.zeros((P, 3), dtype=np.float32)
    wb = np.full((P, 3), -1e30, dtype=np.float32)
    for p in range(P):
        g = p // 16
        for d in range(3):
            c = 3 * g + d
            if g < NPL and c < NCLS:
                wsc[p, d] = 64.0 * w[c]
                wb[p, d] = float(-c)
    return lhst_bf, wsc, wb


def _build():
    nc = bacc.Bacc(
        "TRN2",
        target_bir_lowering=False,
        debug=False,
        num_devices=NCORES,
    )
    v_d = nc.dram_tensor("v", [P, FREE], _DT.bfloat16, kind="ExternalInput").ap()
    lhst_d = nc.dram_tensor("lhst", [P, NPL * P], _DT.bfloat16, kind="ExternalInput").ap()
    wsc_d = nc.dram_tensor("wsc", [P, 3], _DT.float32, kind="ExternalInput").ap()
    wb_d = nc.dram_tensor("wb", [P, 3], _DT.float32, kind="ExternalInput").ap()
    out_d = nc.dram_tensor("out", [16, 1024], _DT.int32, kind="ExternalOutput").ap()

    with tile.TileContext(nc) as tc:
        with (
            tc.tile_pool(name="const", bufs=1) as cpool,
            tc.tile_pool(name="v", bufs=1) as vpool,
            tc.tile_pool(name="pl", bufs=3) as plpool,
            tc.tile_pool(name="psum", bufs=1, space="PSUM") as ppool,
            tc.tile_pool(name="dec", bufs=1) as dpool,
        ):
            lhst = cpool.tile([P, NPL * P], _DT.bfloat16)
            nc.scalar.dma_start(out=lhst[:], in_=lhst_d)
            wsc = cpool.tile([P, 3], _DT.float32)
            nc.scalar.dma_start(out=wsc[:], in_=wsc_d)
            wb = cpool.tile([P, 3], _DT.float32)
            nc.scalar.dma_start(out=wb[:], in_=wb_d)

            vt = vpool.tile([P, FREE], _DT.bfloat16)
            for k in range(2):
                nc.sync.dma_start(out=vt[:, k * 4096:(k + 1) * 4096],
                                  in_=v_d[:, k * 4096:(k + 1) * 4096])

            ps = ppool.tile([P, 1024], _DT.float32)

            for g in range(NPL):
                pl = plpool.tile([P, FREE], _DT.bfloat16, name="pl", tag="pl")
                cap = float(2.0 ** (8 * (3 * g + 2) - 100))
                scl = float(2.0 ** (100 - 24 * g))
                nc.vector.tensor_scalar(
                    out=pl[:], in0=vt[:],
                    scalar1=cap, scalar2=scl,
                    op0=mybir.AluOpType.min, op1=mybir.AluOpType.mult,
                )
                lt = lhst[:, g * P:(g + 1) * P]
                pl3 = pl[:].rearrange("p (r c n) -> p r c n", r=DSF, c=2)
                for r in range(DSF):
                    for c in range(2):
                        nc.tensor.matmul(
                            ps[:, c * 512:(c + 1) * 512],
                            lt,
                            pl3[:, r, c],
                            start=(g == 0 and r == 0),
                            stop=(g == NPL - 1 and r == DSF - 1),
                        )

            # ---- decode tail ----
            c2i = dpool.tile([P, 1024], _DT.int32, name="c2i")
            nc.vector.tensor_scalar(
                out=c2i[:], in0=ps[:], scalar1=float(2.0 ** -16),
                scalar2=None, op0=mybir.AluOpType.mult,
            )
            r_t = dpool.tile([P, 1024], _DT.float32, name="r_t")
            nc.vector.scalar_tensor_tensor(
                out=r_t[:], in0=c2i[:], scalar=-65536.0, in1=ps[:],
                op0=mybir.AluOpType.mult, op1=mybir.AluOpType.add,
            )
            c1i = dpool.tile([P, 1024], _DT.int32, name="c1i")
            nc.vector.tensor_scalar(
                out=c1i[:], in0=r_t[:], scalar1=float(2.0 ** -8),
                scalar2=None, op0=mybir.AluOpType.mult,
            )
            c0f = dpool.tile([P, 1024], _DT.float32, name="c0f")
            nc.vector.scalar_tensor_tensor(
                out=c0f[:], in0=c1i[:], scalar=-256.0, in1=r_t[:],
                op0=mybir.AluOpType.mult, op1=mybir.AluOpType.add,
            )
            c0i = dpool.tile([P, 1024], _DT.int32, name="c0i")
            nc.vector.tensor_scalar(
                out=c0i[:], in0=c0f[:], scalar1=1.0,
                scalar2=None, op0=mybir.AluOpType.mult,
            )
            # stripe digit-sums D = c0 + c1 + c2' (on gpsimd to offload DVE)
            s01 = dpool.tile([P, 1024], _DT.int32, name="s01")
            nc.gpsimd.tensor_tensor(out=s01[:], in0=c0i[:], in1=c1i[:],
                                    op=mybir.AluOpType.add)
            dsum = dpool.tile([P, 1024], _DT.int32, name="dsum")
            nc.gpsimd.tensor_tensor(out=dsum[:], in0=s01[:], in1=c2i[:],
                                    op=mybir.AluOpType.add)
            # T_g = digit-sum of plane g+1: partition shift by 16
            sd = dpool.tile([112, 1024], _DT.int32, name="sd")
            nc.sync.dma_start(out=sd[:], in_=dsum[16:128, :])
            ct = dpool.tile([112, 1024], _DT.int32, name="ct")
            nc.vector.tensor_tensor(out=ct[:], in0=c2i[0:112, :], in1=sd[:],
                                    op=mybir.AluOpType.subtract)
            # E = 64*w*c - cls via ACT Identity with per-partition APs
            e0 = dpool.tile([P, 1024], _DT.float32, name="e0")
            nc.scalar.activation(e0[:], c0i[:],
                                 mybir.ActivationFunctionType.Identity,
                                 bias=wb[:, 0:1], scale=wsc[:, 0:1])
            e1 = dpool.tile([P, 1024], _DT.float32, name="e1")
            nc.scalar.activation(e1[:], c1i[:],
                                 mybir.ActivationFunctionType.Identity,
                                 bias=wb[:, 1:2], scale=wsc[:, 1:2])
            e2 = dpool.tile([112, 1024], _DT.float32, name="e2")
            nc.scalar.activation(e2[:], ct[:],
                                 mybir.ActivationFunctionType.Identity,
                                 bias=wb[0:112, 2:3], scale=wsc[0:112, 2:3])
            m01 = dpool.tile([P, 1024], _DT.float32, name="m01")
            nc.vector.tensor_tensor(out=m01[:], in0=e0[:], in1=e1[:],
                                    op=mybir.AluOpType.max)
            m = dpool.tile([112, 1024], _DT.float32, name="m")
            nc.vector.tensor_tensor(out=m[:], in0=m01[0:112, :], in1=e2[:],
                                    op=mybir.AluOpType.max)
            # fold 7 stripes of 16 partitions down to 1 stripe
            # (engine APs must start at partition 0; DMA moves are exempt)
            mv48 = dpool.tile([48, 1024], _DT.float32, name="mv48")
            nc.sync.dma_start(out=mv48[:], in_=m[64:112, :])
            fa = dpool.tile([48, 1024], _DT.float32, name="fa")
            nc.vector.tensor_tensor(out=fa[:], in0=m[0:48, :],
                                    in1=mv48[:], op=mybir.AluOpType.max)
            mvb = dpool.tile([32, 1024], _DT.float32, name="mvb")
            nc.sync.dma_start(out=mvb[0:16, :], in_=fa[32:48, :])
            nc.sync.dma_start(out=mvb[16:32, :], in_=m[48:64, :])
            fb = dpool.tile([32, 1024], _DT.float32, name="fb")
            nc.vector.tensor_tensor(out=fb[:], in0=fa[0:32, :],
                                    in1=mvb[:], op=mybir.AluOpType.max)
            mvc = dpool.tile([16, 1024], _DT.float32, name="mvc")
            nc.sync.dma_start(out=mvc[:], in_=fb[16:32, :])
            cur = dpool.tile([16, 1024], _DT.float32, name="fc")
            nc.vector.tensor_tensor(out=cur[:], in0=fb[0:16, :],
                                    in1=mvc[:], op=mybir.AluOpType.max)
            # decode: wi = (F+25)/64 (trunc/round agree); cls = 64*wi - F
            wi = dpool.tile([16, 1024], _DT.int32, name="wi")
            nc.vector.tensor_scalar(
                out=wi[:], in0=cur[:], scalar1=25.0, scalar2=1.0 / 64.0,
                op0=mybir.AluOpType.add, op1=mybir.AluOpType.mult,
            )
            out_t = dpool.tile([16, 1024], _DT.int32, name="out_t")
            nc.vector.scalar_tensor_tensor(
                out=out_t[:], in0=wi[:], scalar=64.0, in1=cur[:],
                op0=mybir.AluOpType.mult, op1=mybir.AluOpType.subtract,
            )
            nc.sync.dma_start(out=out_d, in_=out_t[:])
    nc.finalize()
    return nc


_CACHED = None


def _get_nc():
    global _CACHED
    if _CACHED is None:
        _CACHED = _build()
    return _CACHED


def kernel(labels: np.ndarray, class_weights: np.ndarray, dsf) -> np.ndarray:
    global LAST_RESULTS
    dsf = int(np.asarray(dsf))
    assert dsf == DSF, f"kernel hardcodes dsf=8, got {dsf}"
    labels = np.asarray(labels)
    out_dtype = labels.dtype
    cw = np.asarray(class_weights, dtype=np.float32)

    lab = labels.reshape(B * H, W).astype(np.int32)
    lhst_bf, wsc, wb = _aux_arrays(cw)
    in_maps = []
    for k in range(NCORES):
        shard = lab[k * ROWS:(k + 1) * ROWS]                    # [512, 2048]
        # [prow, r, wc, p] -> [p, r, wc, prow]
        xi = shard.reshape(PROWS, DSF, WC, P).transpose(3, 1, 2, 0)
        v = np.ldexp(np.float32(1.0), 8 * xi - 100)
        v = np.ascontiguousarray(v).astype(ml_dtypes.bfloat16).reshape(P, FREE)
        in_maps.append({"v": v, "lhst": lhst_bf, "wsc": wsc, "wb": wb})

    nc = _get_nc()
    res = run_bass_kernel_spmd(
        nc, in_maps, core_ids=list(range(NCORES)), trace=TRACE,
    )
    LAST_RESULTS = res

    # out[pi, wc*64 + prow] -> modes[64k + prow, wc*16 + pi]
    modes = np.empty((B * GH, GW), dtype=np.int64)
    for k in range(NCORES):
        o = res.results[k]["out"].reshape(16, WC, PROWS)
        blk = o.transpose(2, 1, 0).reshape(PROWS, GW)           # [prow, wc*16+pi]
        modes[k * PROWS:(k + 1) * PROWS] = blk
    return modes.reshape(B, GH, GW).astype(out_dtype)


# revision 7
# speedup vs baseline: 1.0254x; 1.0254x over previous
"""ClassWeightedModalDownSampler Trainium2 kernel — packed-digit histogram.

labels [4,1024,2048] (0..19), class_weights [20] f32, dsf=8 ->
modes [4,128,256]: per 8x8 patch argmax_c(hist_c * w_c), first-index ties.

Scheme (8 cores, 64 patch rows each):
  host:  V[p, n] = 2^(8*l - 100) as bf16 (exact powers of two), p = w mod 128,
         n = r*1024 + wc*64 + prow.
  planes: 7 planes of 3 classes {3g, 3g+1, 3g+2} built on DVE at 4x:
         P_g = min(V, 2^(8*(3g+2)-100)) * 2^(100-24g): in-window pixels give
         digits {1, 2^8, 2^16}; below-window pixels leave <=2^-8 crumbs;
         above-window pixels CLAMP onto the top digit (corrected later).
  PE:    per plane one pooling pass: block-ones lhsT contracts each patch's
         8 q-columns, 8 accumulating matmuls contract the 8 rows ->
         PSUM [128, 1024]: partition 16g+pi holds c0 + c1*256 + c2'*65536,
         with c2' = c2 + T_g (T_g = # pixels with class >= 3g+3).
  tail:  digit peel (int32 casts kill crumbs); T_g = digit-sum of plane g+1
         (telescoping), fetched by a 16-partition DMA shift; per-partition
         E = 64*w*c - cls on ACT; max across digits, stripe folds, decode.
"""

import numpy as np
import ml_dtypes

import concourse.bass as bass
import concourse.mybir as mybir
import concourse.tile as tile
from concourse import bacc
from concourse.bass_utils import run_bass_kernel_spmd

NCORES = 8
B, H, W = 4, 1024, 2048
DSF = 8
NCLS = 20
GH, GW = H // DSF, W // DSF
ROWS = (B * H) // NCORES     # 512 label rows per core
PROWS = ROWS // DSF          # 64 patch rows per core
P = 128
WC = W // P                  # 16 column chunks of 128
FREE = WC * ROWS             # 8192 free positions per partition
NPL = 7                      # planes of 3 classes (plane 6: 18, 19, dummy)

_DT = mybir.dt

TRACE = False
LAST_RESULTS = None


def _aux_arrays(class_weights: np.ndarray):
    # lhsT blocks: 7 matrices [128,128] bf16; block g has ones at
    # (p, 16*g + p//8): contracts each patch's 8 q-columns into
    # output partition 16*g + patch-index.
    lhst = np.zeros((P, NPL * P), dtype=np.float32)
    for g in range(NPL):
        for p in range(P):
            lhst[p, g * P + 16 * g + p // 8] = 1.0
    lhst_bf = lhst.astype(ml_dtypes.bfloat16)

    # per-partition E-encode constants: partition p = 16*g + pi, digit d
    # -> class c = 3g + d: scale 64*w_c, bias -c. Dummy slots lose the max.
    w = np.asarray(class_weights, dtype=np.float32)
    wsc = np.zeros((P, 3), dtype=np.float32)
    wb = np.full((P, 3), -1e30, dtype=np.float32)
    for p in range(P):
        g = p // 16
        for d in range(3):
            c = 3 * g + d
            if g < NPL and c < NCLS:
                wsc[p, d] = 64.0 * w[c]
                wb[p, d] = float(-c)
    return lhst_bf, wsc, wb


def _build():
    nc = bacc.Bacc(
        "TRN2",
        target_bir_lowering=False,
        debug=False,
        num_devices=NCORES,
    )
    v_d = nc.dram_tensor("v", [P, FREE], _DT.bfloat16, kind="ExternalInput").ap()
    lhst_d = nc.dram_tensor("lhst", [P, NPL * P], _DT.bfloat16, kind="ExternalInput").ap()
    wsc_d = nc.dram_tensor("wsc", [P, 3], _DT.float32, kind="ExternalInput").ap()
    wb_d = nc.dram_tensor("wb", [P, 3], _DT.float32, kind="ExternalInput").ap()
    out_d = nc.dram_tensor("out", [16, 1024], _DT.int32, kind="ExternalOutput").ap()

    with tile.TileContext(nc) as tc:
        with (
            tc.tile_pool(name="const", bufs=1) as cpool,
            tc.tile_pool(name="v", bufs=1) as vpool,
            tc.tile_pool(name="pl", bufs=3) as plpool,
            tc.tile_pool(name="psum", bufs=1, space="PSUM") as ppool,
            tc.tile_pool(name="dec", bufs=1) as dpool,
        ):
            lhst = cpool.tile([P, NPL * P], _DT.bfloat16)
            nc.scalar.dma_start(out=lhst[:], in_=lhst_d)
            wsc = cpool.tile([P, 3], _DT.float32)
            nc.scalar.dma_start(out=wsc[:], in_=wsc_d)
            wb = cpool.tile([P, 3], _DT.float32)
            nc.scalar.dma_start(out=wb[:], in_=wb_d)

            vt = vpool.tile([P, FREE], _DT.bfloat16)
            for k in range(2):
                nc.sync.dma_start(out=vt[:, k * 4096:(k + 1) * 4096],
                                  in_=v_d[:, k * 4096:(k + 1) * 4096])

            ps = ppool.tile([P, 1024], _DT.float32)

            for g in range(NPL):
                pl = plpool.tile([P, FREE], _DT.bfloat16, name="pl", tag="pl")
                cap = float(2.0 ** (8 * (3 * g + 2) - 100))
                scl = float(2.0 ** (100 - 24 * g))
                nc.vector.tensor_scalar(
                    out=pl[:], in0=vt[:],
                    scalar1=cap, scalar2=scl,
                    op0=mybir.AluOpType.min, op1=mybir.AluOpType.mult,
                )
                lt = lhst[:, g * P:(g + 1) * P]
                pl3 = pl[:].rearrange("p (r c n) -> p r c n", r=DSF, c=2)
                for r in range(DSF):
                    for c in range(2):
                        nc.tensor.matmul(
                            ps[:, c * 512:(c + 1) * 512],
                            lt,
                            pl3[:, r, c],
                            start=(g == 0 and r == 0),
                            stop=(g == NPL - 1 and r == DSF - 1),
                        )

            # ---- decode tail ----
            c2i = dpool.tile([P, 1024], _DT.int32, name="c2i")
            nc.vector.tensor_scalar(
                out=c2i[:], in0=ps[:], scalar1=float(2.0 ** -16),
                scalar2=None, op0=mybir.AluOpType.mult,
            )
            r_t = dpool.tile([P, 1024], _DT.float32, name="r_t")
            nc.vector.scalar_tensor_tensor(
                out=r_t[:], in0=c2i[:], scalar=-65536.0, in1=ps[:],
                op0=mybir.AluOpType.mult, op1=mybir.AluOpType.add,
            )
            c1i = dpool.tile([P, 1024], _DT.int32, name="c1i")
            nc.vector.tensor_scalar(
                out=c1i[:], in0=r_t[:], scalar1=float(2.0 ** -8),
                scalar2=None, op0=mybir.AluOpType.mult,
            )
            c0f = dpool.tile([P, 1024], _DT.float32, name="c0f")
            nc.vector.scalar_tensor_tensor(
                out=c0f[:], in0=c1i[:], scalar=-256.0, in1=r_t[:],
                op0=mybir.AluOpType.mult, op1=mybir.AluOpType.add,
            )
            c0i = dpool.tile([P, 1024], _DT.int32, name="c0i")
            nc.vector.tensor_scalar(
                out=c0i[:], in0=c0f[:], scalar1=1.0,
                scalar2=None, op0=mybir.AluOpType.mult,
            )
            # stripe digit-sums D = c0 + c1 + c2'; c0+c1 = trunc(R*2^-8)+R
            # is wrong, so just two adds; keep them on DVE (GPSIMD adds are
            # 0.42-efficiency and sit on the critical path).
            s01 = dpool.tile([P, 1024], _DT.int32, name="s01")
            nc.vector.tensor_tensor(out=s01[:], in0=c0i[:], in1=c1i[:],
                                    op=mybir.AluOpType.add)
            dsum = dpool.tile([P, 1024], _DT.int32, name="dsum")
            nc.vector.tensor_tensor(out=dsum[:], in0=s01[:], in1=c2i[:],
                                    op=mybir.AluOpType.add)
            # T_g = digit-sum of plane g+1: partition shift by 16
            sd = dpool.tile([112, 1024], _DT.int32, name="sd")
            nc.sync.dma_start(out=sd[:], in_=dsum[16:128, :])
            ct = dpool.tile([112, 1024], _DT.int32, name="ct")
            nc.vector.tensor_tensor(out=ct[:], in0=c2i[0:112, :], in1=sd[:],
                                    op=mybir.AluOpType.subtract)
            # E = 64*w*c - cls via ACT Identity with per-partition APs
            e0 = dpool.tile([P, 1024], _DT.float32, name="e0")
            nc.scalar.activation(e0[:], c0i[:],
                                 mybir.ActivationFunctionType.Identity,
                                 bias=wb[:, 0:1], scale=wsc[:, 0:1])
            e1 = dpool.tile([P, 1024], _DT.float32, name="e1")
            nc.scalar.activation(e1[:], c1i[:],
                                 mybir.ActivationFunctionType.Identity,
                                 bias=wb[:, 1:2], scale=wsc[:, 1:2])
            e2 = dpool.tile([112, 1024], _DT.float32, name="e2")
            nc.scalar.activation(e2[:], ct[:],
                                 mybir.ActivationFunctionType.Identity,
                                 bias=wb[0:112, 2:3], scale=wsc[0:112, 2:3])
            m01 = dpool.tile([P, 1024], _DT.float32, name="m01")
            nc.vector.tensor_tensor(out=m01[:], in0=e0[:], in1=e1[:],
                                    op=mybir.AluOpType.max)
            m = dpool.tile([112, 1024], _DT.float32, name="m")
            nc.vector.tensor_tensor(out=m[:], in0=m01[0:112, :], in1=e2[:],
                                    op=mybir.AluOpType.max)
            # fold 7 stripes of 16 partitions down to 1 stripe
            # (engine APs must start at partition 0; DMA moves are exempt)
            mv48 = dpool.tile([48, 1024], _DT.float32, name="mv48")
            nc.sync.dma_start(out=mv48[:], in_=m[64:112, :])
            fa = dpool.tile([48, 1024], _DT.float32, name="fa")
            nc.vector.tensor_tensor(out=fa[:], in0=m[0:48, :],
                                    in1=mv48[:], op=mybir.AluOpType.max)
            mvb = dpool.tile([32, 1024], _DT.float32, name="mvb")
            nc.sync.dma_start(out=mvb[0:16, :], in_=fa[32:48, :])
            nc.sync.dma_start(out=mvb[16:32, :], in_=m[48:64, :])
            fb = dpool.tile([32, 1024], _DT.float32, name="fb")
            nc.vector.tensor_tensor(out=fb[:], in0=fa[0:32, :],
                                    in1=mvb[:], op=mybir.AluOpType.max)
            mvc = dpool.tile([16, 1024], _DT.float32, name="mvc")
            nc.sync.dma_start(out=mvc[:], in_=fb[16:32, :])
            cur = dpool.tile([16, 1024], _DT.float32, name="fc")
            nc.vector.tensor_tensor(out=cur[:], in0=fb[0:16, :],
                                    in1=mvc[:], op=mybir.AluOpType.max)
            # decode: wi = (F+25)/64 (trunc/round agree); cls = 64*wi - F
            wi = dpool.tile([16, 1024], _DT.int32, name="wi")
            nc.vector.tensor_scalar(
                out=wi[:], in0=cur[:], scalar1=25.0, scalar2=1.0 / 64.0,
                op0=mybir.AluOpType.add, op1=mybir.AluOpType.mult,
            )
            out_t = dpool.tile([16, 1024], _DT.int32, name="out_t")
            nc.vector.scalar_tensor_tensor(
                out=out_t[:], in0=wi[:], scalar=64.0, in1=cur[:],
                op0=mybir.AluOpType.mult, op1=mybir.AluOpType.subtract,
            )
            nc.sync.dma_start(out=out_d, in_=out_t[:])
    nc.finalize()
    return nc


_CACHED = None


def _get_nc():
    global _CACHED
    if _CACHED is None:
        _CACHED = _build()
    return _CACHED


def kernel(labels: np.ndarray, class_weights: np.ndarray, dsf) -> np.ndarray:
    global LAST_RESULTS
    dsf = int(np.asarray(dsf))
    assert dsf == DSF, f"kernel hardcodes dsf=8, got {dsf}"
    labels = np.asarray(labels)
    out_dtype = labels.dtype
    cw = np.asarray(class_weights, dtype=np.float32)

    lab = labels.reshape(B * H, W).astype(np.int32)
    lhst_bf, wsc, wb = _aux_arrays(cw)
    in_maps = []
    for k in range(NCORES):
        shard = lab[k * ROWS:(k + 1) * ROWS]                    # [512, 2048]
        # [prow, r, wc, p] -> [p, r, wc, prow]
        xi = shard.reshape(PROWS, DSF, WC, P).transpose(3, 1, 2, 0)
        v = np.ldexp(np.float32(1.0), 8 * xi - 100)
        v = np.ascontiguousarray(v).astype(ml_dtypes.bfloat16).reshape(P, FREE)
        in_maps.append({"v": v, "lhst": lhst_bf, "wsc": wsc, "wb": wb})

    nc = _get_nc()
    res = run_bass_kernel_spmd(
        nc, in_maps, core_ids=list(range(NCORES)), trace=TRACE,
    )
    LAST_RESULTS = res

    # out[pi, wc*64 + prow] -> modes[64k + prow, wc*16 + pi]
    modes = np.empty((B * GH, GW), dtype=np.int64)
    for k in range(NCORES):
        o = res.results[k]["out"].reshape(16, WC, PROWS)
        blk = o.transpose(2, 1, 0).reshape(PROWS, GW)           # [prow, wc*16+pi]
        modes[k * PROWS:(k + 1) * PROWS] = blk
    return modes.reshape(B, GH, GW).astype(out_dtype)


# revision 8
# speedup vs baseline: 1.0279x; 1.0024x over previous
"""ClassWeightedModalDownSampler Trainium2 kernel — packed-digit histogram.

labels [4,1024,2048] (0..19), class_weights [20] f32, dsf=8 ->
modes [4,128,256]: per 8x8 patch argmax_c(hist_c * w_c), first-index ties.

Scheme (8 cores, 64 patch rows each):
  host:  V[p, n] = 2^(8*l - 100) as bf16 (exact powers of two), p = w mod 128,
         n = r*1024 + wc*64 + prow.
  planes: 7 planes of 3 classes {3g, 3g+1, 3g+2} built on DVE at 4x:
         P_g = min(V, 2^(8*(3g+2)-100)) * 2^(100-24g): in-window pixels give
         digits {1, 2^8, 2^16}; below-window pixels leave <=2^-8 crumbs;
         above-window pixels CLAMP onto the top digit (corrected later).
  PE:    per plane one pooling pass: block-ones lhsT contracts each patch's
         8 q-columns, 8 accumulating matmuls contract the 8 rows ->
         PSUM [128, 1024]: partition 16g+pi holds c0 + c1*256 + c2'*65536,
         with c2' = c2 + T_g (T_g = # pixels with class >= 3g+3).
  tail:  digit peel (int32 casts kill crumbs); T_g = digit-sum of plane g+1
         (telescoping), fetched by a 16-partition DMA shift; per-partition
         E = 64*w*c - cls on ACT; max across digits, stripe folds, decode.
"""

import numpy as np
import ml_dtypes

import concourse.bass as bass
import concourse.mybir as mybir
import concourse.tile as tile
from concourse import bacc
from concourse.bass_utils import run_bass_kernel_spmd

NCORES = 8
B, H, W = 4, 1024, 2048
DSF = 8
NCLS = 20
GH, GW = H // DSF, W // DSF
ROWS = (B * H) // NCORES     # 512 label rows per core
PROWS = ROWS // DSF          # 64 patch rows per core
P = 128
WC = W // P                  # 16 column chunks of 128
FREE = WC * ROWS             # 8192 free positions per partition
NPL = 7                      # planes of 3 classes (plane 6: 18, 19, dummy)

_DT = mybir.dt

TRACE = False
LAST_RESULTS = None


def _aux_arrays(class_weights: np.ndarray):
    # lhsT blocks: 7 matrices [128,128] bf16; block g has ones at
    # (p, 16*g + p//8): contracts each patch's 8 q-columns into
    # output partition 16*g + patch-index.
    lhst = np.zeros((P, NPL * P), dtype=np.float32)
    for g in range(NPL):
        for p in range(P):
            lhst[p, g * P + 16 * g + p // 8] = 1.0
    lhst_bf = lhst.astype(ml_dtypes.bfloat16)

    # per-partition E-encode constants: partition p = 16*g + pi, digit d
    # -> class c = 3g + d: scale 64*w_c, bias -c. Dummy slots lose the max.
    w = np.asarray(class_weights, dtype=np.float32)
    wsc = np.zeros((P, 3), dtype=np.float32)
    wb = np.full((P, 3), -1e30, dtype=np.float32)
    for p in range(P):
        g = p // 16
        for d in range(3):
            c = 3 * g + d
            if g < NPL and c < NCLS:
                wsc[p, d] = 64.0 * w[c]
                wb[p, d] = float(-c)
    return lhst_bf, wsc, wb


def _build():
    nc = bacc.Bacc(
        "TRN2",
        target_bir_lowering=False,
        debug=False,
        num_devices=NCORES,
    )
    v_d = nc.dram_tensor("v", [P, FREE], _DT.bfloat16, kind="ExternalInput").ap()
    lhst_d = nc.dram_tensor("lhst", [P, NPL * P], _DT.bfloat16, kind="ExternalInput").ap()
    wsc_d = nc.dram_tensor("wsc", [P, 3], _DT.float32, kind="ExternalInput").ap()
    wb_d = nc.dram_tensor("wb", [P, 3], _DT.float32, kind="ExternalInput").ap()
    out_d = nc.dram_tensor("out", [16, 1024], _DT.int32, kind="ExternalOutput").ap()

    with tile.TileContext(nc) as tc:
        with (
            tc.tile_pool(name="const", bufs=1) as cpool,
            tc.tile_pool(name="v", bufs=1) as vpool,
            tc.tile_pool(name="pl", bufs=3) as plpool,
            tc.tile_pool(name="psum", bufs=1, space="PSUM") as ppool,
            tc.tile_pool(name="dec", bufs=1) as dpool,
        ):
            lhst = cpool.tile([P, NPL * P], _DT.bfloat16)
            nc.scalar.dma_start(out=lhst[:], in_=lhst_d)
            wsc = cpool.tile([P, 3], _DT.float32)
            nc.scalar.dma_start(out=wsc[:], in_=wsc_d)
            wb = cpool.tile([P, 3], _DT.float32)
            nc.scalar.dma_start(out=wb[:], in_=wb_d)

            vt = vpool.tile([P, FREE], _DT.bfloat16)
            for k in range(2):
                nc.sync.dma_start(out=vt[:, k * 4096:(k + 1) * 4096],
                                  in_=v_d[:, k * 4096:(k + 1) * 4096])

            ps = ppool.tile([P, 1024], _DT.float32)

            # warm the PE p-state during the input DMA: dummy matmuls on the
            # (already-loaded, tiny) lhst tile keep the ramp model at full
            # clock by the time the real pooling chain starts.
            warm = ppool.tile([P, 512], _DT.float32, name="warm", tag="warm")
            for i in range(24):
                nc.tensor.matmul(warm[:], lhst[:, 0:P], lhst[:, 256:768],
                                 start=True, stop=True)

            for g in range(NPL):
                pl = plpool.tile([P, FREE], _DT.bfloat16, name="pl", tag="pl")
                cap = float(2.0 ** (8 * (3 * g + 2) - 100))
                scl = float(2.0 ** (100 - 24 * g))
                nc.vector.tensor_scalar(
                    out=pl[:], in0=vt[:],
                    scalar1=cap, scalar2=scl,
                    op0=mybir.AluOpType.min, op1=mybir.AluOpType.mult,
                )
                lt = lhst[:, g * P:(g + 1) * P]
                pl3 = pl[:].rearrange("p (r c n) -> p r c n", r=DSF, c=2)
                for r in range(DSF):
                    for c in range(2):
                        nc.tensor.matmul(
                            ps[:, c * 512:(c + 1) * 512],
                            lt,
                            pl3[:, r, c],
                            start=(g == 0 and r == 0),
                            stop=(g == NPL - 1 and r == DSF - 1),
                        )

            # ---- decode tail ----
            c2i = dpool.tile([P, 1024], _DT.int32, name="c2i")
            nc.vector.tensor_scalar(
                out=c2i[:], in0=ps[:], scalar1=float(2.0 ** -16),
                scalar2=None, op0=mybir.AluOpType.mult,
            )
            r_t = dpool.tile([P, 1024], _DT.float32, name="r_t")
            nc.vector.scalar_tensor_tensor(
                out=r_t[:], in0=c2i[:], scalar=-65536.0, in1=ps[:],
                op0=mybir.AluOpType.mult, op1=mybir.AluOpType.add,
            )
            c1i = dpool.tile([P, 1024], _DT.int32, name="c1i")
            nc.vector.tensor_scalar(
                out=c1i[:], in0=r_t[:], scalar1=float(2.0 ** -8),
                scalar2=None, op0=mybir.AluOpType.mult,
            )
            c0f = dpool.tile([P, 1024], _DT.float32, name="c0f")
            nc.vector.scalar_tensor_tensor(
                out=c0f[:], in0=c1i[:], scalar=-256.0, in1=r_t[:],
                op0=mybir.AluOpType.mult, op1=mybir.AluOpType.add,
            )
            c0i = dpool.tile([P, 1024], _DT.int32, name="c0i")
            nc.vector.tensor_scalar(
                out=c0i[:], in0=c0f[:], scalar1=1.0,
                scalar2=None, op0=mybir.AluOpType.mult,
            )
            # stripe digit-sums D = c0 + c1 + c2'; c0+c1 = trunc(R*2^-8)+R
            # is wrong, so just two adds; keep them on DVE (GPSIMD adds are
            # 0.42-efficiency and sit on the critical path).
            s01 = dpool.tile([P, 1024], _DT.int32, name="s01")
            nc.vector.tensor_tensor(out=s01[:], in0=c0i[:], in1=c1i[:],
                                    op=mybir.AluOpType.add)
            dsum = dpool.tile([P, 1024], _DT.int32, name="dsum")
            nc.vector.tensor_tensor(out=dsum[:], in0=s01[:], in1=c2i[:],
                                    op=mybir.AluOpType.add)
            # T_g = digit-sum of plane g+1: partition shift by 16
            sd = dpool.tile([112, 1024], _DT.int32, name="sd")
            nc.sync.dma_start(out=sd[:], in_=dsum[16:128, :])
            ct = dpool.tile([112, 1024], _DT.int32, name="ct")
            nc.vector.tensor_tensor(out=ct[:], in0=c2i[0:112, :], in1=sd[:],
                                    op=mybir.AluOpType.subtract)
            # E = 64*w*c - cls via ACT Identity with per-partition APs
            e0 = dpool.tile([P, 1024], _DT.float32, name="e0")
            nc.scalar.activation(e0[:], c0i[:],
                                 mybir.ActivationFunctionType.Identity,
                                 bias=wb[:, 0:1], scale=wsc[:, 0:1])
            e1 = dpool.tile([P, 1024], _DT.float32, name="e1")
            nc.scalar.activation(e1[:], c1i[:],
                                 mybir.ActivationFunctionType.Identity,
                                 bias=wb[:, 1:2], scale=wsc[:, 1:2])
            e2 = dpool.tile([112, 1024], _DT.float32, name="e2")
            nc.scalar.activation(e2[:], ct[:],
                                 mybir.ActivationFunctionType.Identity,
                                 bias=wb[0:112, 2:3], scale=wsc[0:112, 2:3])
            m01 = dpool.tile([P, 1024], _DT.float32, name="m01")
            nc.vector.tensor_tensor(out=m01[:], in0=e0[:], in1=e1[:],
                                    op=mybir.AluOpType.max)
            m = dpool.tile([112, 1024], _DT.float32, name="m")
            nc.vector.tensor_tensor(out=m[:], in0=m01[0:112, :], in1=e2[:],
                                    op=mybir.AluOpType.max)
            # fold 7 stripes of 16 partitions down to 1 stripe
            # (engine APs must start at partition 0; DMA moves are exempt)
            mv48 = dpool.tile([48, 1024], _DT.float32, name="mv48")
            nc.sync.dma_start(out=mv48[:], in_=m[64:112, :])
            fa = dpool.tile([48, 1024], _DT.float32, name="fa")
            nc.vector.tensor_tensor(out=fa[:], in0=m[0:48, :],
                                    in1=mv48[:], op=mybir.AluOpType.max)
            mvb = dpool.tile([32, 1024], _DT.float32, name="mvb")
            nc.sync.dma_start(out=mvb[0:16, :], in_=fa[32:48, :])
            nc.sync.dma_start(out=mvb[16:32, :], in_=m[48:64, :])
            fb = dpool.tile([32, 1024], _DT.float32, name="fb")
            nc.vector.tensor_tensor(out=fb[:], in0=fa[0:32, :],
                                    in1=mvb[:], op=mybir.AluOpType.max)
            mvc = dpool.tile([16, 1024], _DT.float32, name="mvc")
            nc.sync.dma_start(out=mvc[:], in_=fb[16:32, :])
            cur = dpool.tile([16, 1024], _DT.float32, name="fc")
            nc.vector.tensor_tensor(out=cur[:], in0=fb[0:16, :],
                                    in1=mvc[:], op=mybir.AluOpType.max)
            # decode: wi = (F+25)/64 (trunc/round agree); cls = 64*wi - F
            wi = dpool.tile([16, 1024], _DT.int32, name="wi")
            nc.vector.tensor_scalar(
                out=wi[:], in0=cur[:], scalar1=25.0, scalar2=1.0 / 64.0,
                op0=mybir.AluOpType.add, op1=mybir.AluOpType.mult,
            )
            out_t = dpool.tile([16, 1024], _DT.int32, name="out_t")
            nc.vector.scalar_tensor_tensor(
                out=out_t[:], in0=wi[:], scalar=64.0, in1=cur[:],
                op0=mybir.AluOpType.mult, op1=mybir.AluOpType.subtract,
            )
            nc.sync.dma_start(out=out_d, in_=out_t[:])
    nc.finalize()
    return nc


_CACHED = None


def _get_nc():
    global _CACHED
    if _CACHED is None:
        _CACHED = _build()
    return _CACHED


def kernel(labels: np.ndarray, class_weights: np.ndarray, dsf) -> np.ndarray:
    global LAST_RESULTS
    dsf = int(np.asarray(dsf))
    assert dsf == DSF, f"kernel hardcodes dsf=8, got {dsf}"
    labels = np.asarray(labels)
    out_dtype = labels.dtype
    cw = np.asarray(class_weights, dtype=np.float32)

    lab = labels.reshape(B * H, W).astype(np.int32)
    lhst_bf, wsc, wb = _aux_arrays(cw)
    in_maps = []
    for k in range(NCORES):
        shard = lab[k * ROWS:(k + 1) * ROWS]                    # [512, 2048]
        # [prow, r, wc, p] -> [p, r, wc, prow]
        xi = shard.reshape(PROWS, DSF, WC, P).transpose(3, 1, 2, 0)
        v = np.ldexp(np.float32(1.0), 8 * xi - 100)
        v = np.ascontiguousarray(v).astype(ml_dtypes.bfloat16).reshape(P, FREE)
        in_maps.append({"v": v, "lhst": lhst_bf, "wsc": wsc, "wb": wb})

    nc = _get_nc()
    res = run_bass_kernel_spmd(
        nc, in_maps, core_ids=list(range(NCORES)), trace=TRACE,
    )
    LAST_RESULTS = res

    # out[pi, wc*64 + prow] -> modes[64k + prow, wc*16 + pi]
    modes = np.empty((B * GH, GW), dtype=np.int64)
    for k in range(NCORES):
        o = res.results[k]["out"].reshape(16, WC, PROWS)
        blk = o.transpose(2, 1, 0).reshape(PROWS, GW)           # [prow, wc*16+pi]
        modes[k * PROWS:(k + 1) * PROWS] = blk
    return modes.reshape(B, GH, GW).astype(out_dtype)


# revision 9
# speedup vs baseline: 1.0887x; 1.0591x over previous
"""ClassWeightedModalDownSampler Trainium2 kernel.

Problem: labels [4, 1024, 2048] int (values 0..19), class_weights [20] f32,
dsf=8.  Output modes [4, 128, 256]: per non-overlapping 8x8 patch, the
argmax over classes of (class histogram * class_weights), first-index
tie-break (jnp.argmax semantics).

Strategy (pure data parallel over 8 cores, each core takes 64 patch rows
= 512 label rows):
  host:  cast labels to bf16 (exact for 0..19) and pre-transpose each
         core's shard to X[p, n] with p = w mod 128 and, per wc-half,
         n = r*512 + wcl*64 + prow (r = row mod 8, prow = row div 8,
         wcl = (w div 128) mod 8), so every matmul rhs is one contiguous
         512-column slice.  Upload is 2 MiB/core.
  compares (one-hot planes, [128, 4096] per wc-half): split across
         DVE (bf16 out at 4x -> normal matmuls; fp8 out at 2x ->
         DoubleRow matmuls), ACT (relu(1-(x-c)^2), exact 0/1 after fp8
         cast), and GPSIMD, balancing engine busy time.
  PE:    per class, PSUM-accumulated matmuls with a block-diagonal ones
         lhsT contract the 8 q-pixels (partition groups of 8) and the 8
         patch rows (8 accumulating matmuls, or 4 fp8 DoubleRow ones)
         -> PSUM [128, 512] banks, 8 classes each: partition 16*j + jj,
         free wcl*64 + prow.
  ACT:   encode E = count * (64*w_c) - c via Identity with per-partition
         scale/bias APs.  Exact for integer class_weights (the contract);
         for fractional weights classes whose fl(w*count) differ by less
         than 19/64 could misrank vs the fp32 reference.
  DVE:   max-fold across banks and partition groups (partition moves via
         SBUF->SBUF DMA), 2-op decode (casts fused), emit int32.
  host:  gather per-core outputs and rearrange to [4, 128, 256].
"""

import numpy as np
import ml_dtypes

import concourse.bass as bass
import concourse.mybir as mybir
import concourse.tile as tile
from concourse import bacc
from concourse.bass_utils import run_bass_kernel_spmd

NCORES = 8
B, H, W = 4, 1024, 2048
DSF = 8
NCLS = 20
GH, GW = H // DSF, W // DSF  # 128, 256 output grid
ROWS = (B * H) // NCORES     # 512 label rows per core
PROWS = ROWS // DSF          # 64 patch rows per core
P = 128
WC = W // P                  # 16 column chunks of 128
HALVES = 2
WCH = WC // HALVES           # 8 chunks per half
FREE = WC * ROWS             # 8192
HFREE = FREE // HALVES       # 4096
NBANK = 3                    # psum banks per half (classes 0-7, 8-15, 16-19x2)

_DT = mybir.dt

# per-class compare engine: spread across DVE (bf16->normal matmul and
# fp8->DoubleRow), ACT (square/relu trick), GPSIMD, to balance busy time.
DVE_BF16 = (0, 1, 2, 3, 8, 9, 10, 11, 16, 17)
DVE_FP8 = (4, 12, 14, 18, 19)
ACT_CLS = (5, 6, 13)
GP_CLS = (7, 15)

# Set by test.py to request a traced run.
TRACE = False
LAST_RESULTS = None


def _aux_arrays(class_weights: np.ndarray):
    """Host-built constant inputs: lhsT matrices and encode scalars."""
    # lhsT blocks: 12 matrices [128, 128] bf16, flattened to [128, 12*128].
    # j = 0..7: ones at (p, 16*j + p//8).   (classes c with c%8 == j)
    # j = 8..11 (jd = j-8): doubled block for classes 16..19:
    #   ones at (p, 16*jd + p//8) and (p, 64 + 16*jd + p//8).
    lhst = np.zeros((P, 12 * P), dtype=np.float32)
    for j in range(8):
        for p in range(P):
            lhst[p, j * P + 16 * j + p // 8] = 1.0
    for jd in range(4):
        for p in range(P):
            base = (8 + jd) * P
            lhst[p, base + 16 * jd + p // 8] = 1.0
            lhst[p, base + 64 + 16 * jd + p // 8] = 1.0
    lhst_bf = lhst.astype(ml_dtypes.bfloat16)
    # DoubleRow wants lhsT [K, 2, M]: two consecutive [P, 128] copies of
    # each block (k-tile pair as contiguous free blocks).
    l8 = lhst.reshape(P, 12, 1, P)
    l8 = np.concatenate([l8, l8], axis=2).reshape(P, 12 * 2 * P)
    lhst_f8 = l8.astype(ml_dtypes.float8_e4m3)

    # ACT compare biases: Square pass bias -c, Relu pass bias 1.0
    actb = np.zeros((P, 2 * len(ACT_CLS)), dtype=np.float32)
    for i, c in enumerate(ACT_CLS):
        actb[:, 2 * i] = -float(c)
        actb[:, 2 * i + 1] = 1.0

    # encode scalars per psum bank g (E = 64*w_c*count - c):
    # partition p holds class
    #   g<2:  c = 8*g + p//16
    #   g==2: c = 16 + (p//16) % 4
    wscale = np.zeros((P, NBANK), dtype=np.float32)
    wbias = np.zeros((P, NBANK), dtype=np.float32)
    w = np.asarray(class_weights, dtype=np.float32)
    for g in range(NBANK):
        for p in range(P):
            c = 8 * g + p // 16 if g < 2 else 16 + (p // 16) % 4
            wscale[p, g] = 64.0 * w[c]
            wbias[p, g] = float(-c)
    return lhst_bf, lhst_f8, actb, wscale, wbias


def _build():
    """Build the SPMD Bass kernel (same NEFF on all 8 cores)."""
    nc = bacc.Bacc(
        "TRN2",
        target_bir_lowering=False,
        debug=False,
        num_devices=NCORES,
    )
    x_d = nc.dram_tensor("x", [P, FREE], _DT.bfloat16, kind="ExternalInput").ap()
    lhstb_d = nc.dram_tensor("lhstb", [P, 12 * P], _DT.bfloat16, kind="ExternalInput").ap()
    lhst8_d = nc.dram_tensor("lhst8", [P, 2 * 12 * P], _DT.float8e4, kind="ExternalInput").ap()
    actb_d = nc.dram_tensor("actb", [P, 2 * len(ACT_CLS)], _DT.float32, kind="ExternalInput").ap()
    wscale_d = nc.dram_tensor("wscale", [P, NBANK], _DT.float32, kind="ExternalInput").ap()
    wbias_d = nc.dram_tensor("wbias", [P, NBANK], _DT.float32, kind="ExternalInput").ap()
    out_d = nc.dram_tensor("out", [16, HALVES * 512], _DT.int32, kind="ExternalOutput").ap()

    with tile.TileContext(nc) as tc:
        with (
            tc.tile_pool(name="const", bufs=1) as cpool,
            tc.tile_pool(name="x", bufs=2) as xpool,
            tc.tile_pool(name="oh", bufs=6) as opool,
            tc.tile_pool(name="psum", bufs=2, space="PSUM") as ppool,
            tc.tile_pool(name="enc", bufs=4) as epool,
            tc.tile_pool(name="small", bufs=4) as spool,
            tc.tile_pool(name="sqp", bufs=2) as sqpool,
            tc.tile_pool(name="outp", bufs=1) as outpool,
        ):
            # consts ride the SWDGE queue so the x DMAs own the sync queue
            lhstb = cpool.tile([P, 12 * P], _DT.bfloat16)
            nc.gpsimd.dma_start(out=lhstb[:], in_=lhstb_d)
            lhst8 = cpool.tile([P, 2 * 12 * P], _DT.float8e4)
            nc.gpsimd.dma_start(out=lhst8[:], in_=lhst8_d)
            actb = cpool.tile([P, 2 * len(ACT_CLS)], _DT.float32)
            nc.gpsimd.dma_start(out=actb[:], in_=actb_d)
            wscale = cpool.tile([P, NBANK], _DT.float32)
            nc.gpsimd.dma_start(out=wscale[:], in_=wscale_d)
            wbias = cpool.tile([P, NBANK], _DT.float32)
            nc.gpsimd.dma_start(out=wbias[:], in_=wbias_d)

            out_t = outpool.tile([16, HALVES * 512], _DT.int32)

            for hf in range(HALVES):
                xt = xpool.tile([P, HFREE], _DT.bfloat16)
                nc.sync.dma_start(out=xt[:], in_=x_d[:, hf * HFREE:(hf + 1) * HFREE])

                banks = [
                    ppool.tile([P, 512], _DT.float32, name=f"bank{g}", tag=f"bank{g}")
                    for g in range(NBANK)
                ]
                # per bank, emit DVE classes first so chains start promptly
                bank_cls = [[], [], []]
                for c in DVE_BF16 + DVE_FP8 + ACT_CLS + GP_CLS:
                    bank_cls[c // 8].append(c)
                for g in range(NBANK):
                    for pos, c in enumerate(bank_cls[g]):
                        j = c % 8
                        first = pos == 0
                        last = pos == len(bank_cls[g]) - 1
                        use_fp8 = c not in DVE_BF16
                        if c in DVE_BF16 or c in DVE_FP8:
                            oh = opool.tile(
                                [P, HFREE],
                                _DT.float8e4 if use_fp8 else _DT.bfloat16,
                                name=f"oh{'8' if use_fp8 else 'b'}",
                                tag=f"oh{'8' if use_fp8 else 'b'}",
                            )
                            nc.vector.tensor_scalar(
                                out=oh[:], in0=xt[:],
                                scalar1=float(c), scalar2=None,
                                op0=mybir.AluOpType.is_equal,
                            )
                        elif c in GP_CLS:
                            oh = opool.tile([P, HFREE], _DT.float8e4,
                                            name="oh8", tag="oh8")
                            nc.gpsimd.tensor_scalar(
                                out=oh[:], in0=xt[:],
                                scalar1=float(c), scalar2=None,
                                op0=mybir.AluOpType.is_equal,
                            )
                        else:  # ACT: relu(1 - (x - c)^2), exact 0/1 after cast
                            ai = ACT_CLS.index(c)
                            sq = sqpool.tile([P, HFREE], _DT.float32,
                                             name="sq", tag="sq")
                            nc.scalar.activation(
                                sq[:], xt[:], mybir.ActivationFunctionType.Square,
                                bias=actb[:, 2 * ai:2 * ai + 1], scale=1.0,
                            )
                            oh = opool.tile([P, HFREE], _DT.float8e4,
                                            name="oh8", tag="oh8")
                            nc.scalar.activation(
                                oh[:], sq[:], mybir.ActivationFunctionType.Relu,
                                bias=actb[:, 2 * ai + 1:2 * ai + 2], scale=-1.0,
                            )
                        # lhsT block: doubled variants for classes 16..19
                        lj = (8 + j) if g == 2 else j
                        if use_fp8:
                            lt = lhst8[:, 2 * lj * P:2 * (lj + 1) * P].rearrange(
                                "p (t m) -> p t m", t=2)
                            oh3 = oh[:].rearrange(
                                "p (rp t n) -> p rp t n",
                                rp=DSF // 2, t=2, n=512)
                            for rp in range(DSF // 2):
                                nc.tensor.matmul(
                                    banks[g][:, :],
                                    lt,
                                    oh3[:, rp],
                                    start=(first and rp == 0),
                                    stop=(last and rp == DSF // 2 - 1),
                                    perf_mode=mybir.MatmulPerfMode.DoubleRow,
                                )
                        else:
                            lt = lhstb[:, lj * P:(lj + 1) * P]
                            for r in range(DSF):
                                nc.tensor.matmul(
                                    banks[g][:, :],
                                    lt,
                                    oh[:, r * 512:(r + 1) * 512],
                                    start=(first and r == 0),
                                    stop=(last and r == DSF - 1),
                                )

                # tail ops jump the engine queues as soon as deps allow
                hp = tc.high_priority()
                hp.__enter__()
                # E = count * (64*w_c) - c   (ACT: per-partition APs)
                encs = []
                for g in range(NBANK):
                    e = epool.tile([P, 512], _DT.float32)
                    nc.scalar.activation(
                        e[:], banks[g][:], mybir.ActivationFunctionType.Identity,
                        bias=wbias[:, g:g + 1], scale=wscale[:, g:g + 1],
                    )
                    encs.append(e)

                # max across banks (same partition layout)
                m01 = epool.tile([P, 512], _DT.float32)
                nc.vector.tensor_tensor(
                    out=m01[:], in0=encs[0][:], in1=encs[1][:],
                    op=mybir.AluOpType.max,
                )
                m = epool.tile([P, 512], _DT.float32)
                nc.vector.tensor_tensor(
                    out=m[:], in0=m01[:], in1=encs[2][:],
                    op=mybir.AluOpType.max,
                )

                # fold partition groups 128 -> 16 (move with DMA, then max)
                cur = m
                for width in (64, 32, 16):
                    moved = spool.tile([width, 512], _DT.float32,
                                       name="moved", tag="moved", bufs=2)
                    nc.sync.dma_start(out=moved[:], in_=cur[width:2 * width, :])
                    nxt = spool.tile([width, 512], _DT.float32,
                                     name="nxt", tag="nxt", bufs=2)
                    nc.vector.tensor_tensor(
                        out=nxt[:], in0=cur[:width, :], in1=moved[:],
                        op=mybir.AluOpType.max,
                    )
                    cur = nxt

                # decode: F = 64W - c; W = cast((F + 25) / 64) (frac =
                # (25 - c)/64 in (0, 0.5): trunc and round-nearest agree);
                # c* = 64W - F, with casts fused into the tensor ops.
                f = cur  # [16, 512] fp32
                wi = spool.tile([16, 512], _DT.int32, name="wi", tag="wi", bufs=2)
                nc.vector.tensor_scalar(
                    out=wi[:], in0=f[:],
                    scalar1=25.0, scalar2=1.0 / 64.0,
                    op0=mybir.AluOpType.add, op1=mybir.AluOpType.mult,
                )
                nc.vector.scalar_tensor_tensor(
                    out=out_t[:, hf * 512:(hf + 1) * 512], in0=wi[:],
                    scalar=64.0, in1=f[:],
                    op0=mybir.AluOpType.mult, op1=mybir.AluOpType.subtract,
                )
                nc.sync.dma_start(
                    out=out_d[:, hf * 512:(hf + 1) * 512],
                    in_=out_t[:, hf * 512:(hf + 1) * 512],
                )
                hp.__exit__(None, None, None)
    nc.finalize()
    return nc


_CACHED = None


def _get_nc():
    global _CACHED
    if _CACHED is None:
        _CACHED = _build()
    return _CACHED


def kernel(labels: np.ndarray, class_weights: np.ndarray, dsf) -> np.ndarray:
    global LAST_RESULTS
    dsf = int(np.asarray(dsf))
    assert dsf == DSF, f"kernel hardcodes dsf=8, got {dsf}"
    labels = np.asarray(labels)
    out_dtype = labels.dtype
    cw = np.asarray(class_weights, dtype=np.float32)

    # host prep: shard rows, extract low 16 bits, transpose to [p, wc*512+row]
    lab = labels.reshape(B * H, W).astype(np.uint16)
    lhst_bf, lhst_f8, actb, wscale, wbias = _aux_arrays(cw)
    in_maps = []
    for k in range(NCORES):
        shard = lab[k * ROWS:(k + 1) * ROWS]                  # [512, 2048]
        # [prow, r, hf, wcl, p] -> [p, hf, r, wcl, prow]
        x = shard.reshape(PROWS, DSF, HALVES, WCH, P).transpose(4, 2, 1, 3, 0)
        x = np.ascontiguousarray(x).astype(ml_dtypes.bfloat16).reshape(P, FREE)
        in_maps.append({
            "x": x,
            "lhstb": lhst_bf,
            "lhst8": lhst_f8,
            "actb": actb,
            "wscale": wscale,
            "wbias": wbias,
        })

    nc = _get_nc()
    res = run_bass_kernel_spmd(
        nc, in_maps, core_ids=list(range(NCORES)), trace=TRACE,
    )
    LAST_RESULTS = res

    # unshard: core k out [16, 1024] int32; out[jj, hf*512 + wcl*64 + prow]
    # -> modes[patch_row = 64k + prow, j = (hf*8 + wcl)*16 + jj]
    modes = np.empty((B * GH, GW), dtype=np.int64)
    for k in range(NCORES):
        o = res.results[k]["out"].reshape(16, HALVES, WCH, PROWS)
        # axes: (jj, hf, wcl, prow) -> [prow, hf, wcl, jj]
        blk = o.transpose(3, 1, 2, 0).reshape(PROWS, WC * 16)
        modes[k * PROWS:(k + 1) * PROWS] = blk
    return modes.reshape(B, GH, GW).astype(out_dtype)

